# revision 21
# baseline (speedup 1.0000x reference)
"""Fused GroupNorm + attention block for Trainium2 (8 NeuronCores, SPMD).

v3 strategy:
  - Each core computes the full output for 1/8 of the spatial positions
    (a 512-column slice of the flattened [C=256, N=4096] activation).
  - x is cast to bf16 on the host (halves the HBM load, no device casts);
    GroupNorm stats run on the bf16 copy; the residual uses bf16 x.
  - GroupNorm folded into the QKV weights on-device; rstd via a
    quake-style rsqrt on the DVE so ACT only ever runs Exp (one table).
  - Attention in "S^T" layout; per head-pair step, softmax exp splits
    across engines: ACT computes exp(s-2) into fp8(e4m3) for the even
    head (shift keeps the range inside fp8; it cancels in normalization),
    the DVE computes the odd head via a Schraudolph exp (one tensor_scalar
    writing bf16 bits as int16, ~3.3% element error that washes out).
  - Even-head AV runs as fp8 DoubleRow matmuls contracting two s-blocks
    (256 positions) per instruction; odd-head AV stays bf16.
  - The attention loop is software-pipelined: the next step's S^T matmul
    is emitted before this step's AV so the PE never waits on exp.
  - Row-sums ride along as an all-ones column of v^T; 1/rowsum via
    reciprocal_approx_fast (base partition 0 - HW quirk), broadcast to
    64 partitions with a tiny K=1 matmul instead of a DMA (the DMA
    completion latency was ~6us).
"""

import numpy as np
import ml_dtypes

import concourse.bass as bass
import concourse.bacc as bacc
import concourse.tile as tile
import concourse.mybir as mybir
from concourse.bass_utils import run_bass_kernel_spmd

F32 = mybir.dt.float32
BF16 = mybir.dt.bfloat16
FP8 = mybir.dt.float8e4
I32 = mybir.dt.int32
I16 = mybir.dt.int16
AF = mybir.ActivationFunctionType
OP = mybir.AluOpType
PM = mybir.MatmulPerfMode

C = 256
N = 4096
NCORES = 8
TSL = N // NCORES          # 512 spatial positions per core
HEADS = 4
D = 64                     # head dim
NG = 16                    # groupnorm groups
GS = C // NG               # channels per group
EPS = 1e-5
NSB = N // 128             # 32 s-blocks
VW = D + 1                 # v^T columns per head incl. ones column
VP8 = 80                   # fp8 v^T padded width (pair step must be %16)
ESH = -2.0                 # logit shift for the fp8 exp path

# Schraudolph exp -> bf16 bits: bits_i16 = trunc(x * SCH_A + SCH_B)
SCH_A = 128.0 / float(np.log(2.0))     # 184.6650558736922
SCH_B = 127.0 * 128.0 - 5.0            # calibrated for truncation


def _build():
    nc = bacc.Bacc("TRN2", target_bir_lowering=False, debug=False,
                   num_devices=NCORES)

    x_d = nc.dram_tensor("xbf", [2, 128, N], BF16, kind="ExternalInput")
    xq_d = nc.dram_tensor("xq", [2, 128, TSL], BF16, kind="ExternalInput")
    wqkvT_d = nc.dram_tensor("wqkvT", [2, 128, 3 * C], BF16, kind="ExternalInput")
    wprojT_d = nc.dram_tensor("wprojT", [2, 128, C], BF16, kind="ExternalInput")
    gamma_d = nc.dram_tensor("gamma_col", [2, 128, 1], F32, kind="ExternalInput")
    beta_d = nc.dram_tensor("beta_col", [2, 128, 1], F32, kind="ExternalInput")
    bproj_d = nc.dram_tensor("bproj_col", [2, 128, 1], F32, kind="ExternalInput")
    g_d = nc.dram_tensor("gmat", [128, NG // 2], F32, kind="ExternalInput")
    gt_d = nc.dram_tensor("gtmat", [NG // 2, 128], F32, kind="ExternalInput")
    out_d = nc.dram_tensor("out", [2, 128, TSL], F32, kind="ExternalOutput")

    with tile.TileContext(nc) as tc:
        _emit(nc, tc, x_d, xq_d, wqkvT_d, wprojT_d, gamma_d, beta_d,
              bproj_d, g_d, gt_d, out_d)
    nc.finalize()
    return nc


def _emit(nc, tc, x_d, xq_d, wqkvT_d, wprojT_d, gamma_d, beta_d, bproj_d,
          g_d, gt_d, out_d):
    import contextlib
    ctx = contextlib.ExitStack()
    with ctx:
        CP = ctx.enter_context(tc.tile_pool(name="const", bufs=1))
        WK = ctx.enter_context(tc.tile_pool(name="work", bufs=2))
        PS = ctx.enter_context(tc.tile_pool(name="psum", bufs=1, space="PSUM"))
        P8Pool = ctx.enter_context(tc.tile_pool(name="p8tiles", bufs=2))
        PBPool = ctx.enter_context(tc.tile_pool(name="pbtiles", bufs=3))

        # ---------------- loads (x bf16 in 8 pieces, two DMA rings) ------
        xch = [[CP.tile([128, N // 4], BF16, tag=f"x{ct}{p}", name=f"x{ct}{p}")
                for p in range(4)] for ct in range(2)]
        NDV = 5            # chunks handled by DVE bn_stats; rest go to ACT
        stats = [WK.tile([128, NDV, 6], F32, tag=f"bnstats{ct}", bufs=1,
                         name=f"bnstats{ct}") for ct in range(2)]
        sxa = [WK.tile([128, 8 - NDV], F32, tag=f"sxa{ct}", bufs=1,
                       name=f"sxa{ct}") for ct in range(2)]
        sxxa = [WK.tile([128, 8 - NDV], F32, tag=f"sxxa{ct}", bufs=1,
                        name=f"sxxa{ct}") for ct in range(2)]
        for p in range(4):
            for ct in range(2):
                eng = nc.sync if ct == 0 else nc.scalar
                eng.dma_start(out=xch[ct][p],
                              in_=x_d[ct, :, p * (N // 4):(p + 1) * (N // 4)])
        xq = [CP.tile([128, TSL], BF16, tag=f"xq{ct}", name=f"xq{ct}")
              for ct in range(2)]
        wqkvT = [CP.tile([128, 3 * C], BF16, tag=f"wq{ct}", name=f"wq{ct}")
                 for ct in range(2)]
        wprojT = [CP.tile([128, C], BF16, tag=f"wp{ct}", name=f"wp{ct}")
                  for ct in range(2)]
        gcol = [CP.tile([128, 1], F32, tag=f"g{ct}", name=f"g{ct}") for ct in range(2)]
        bcol = [CP.tile([128, 1], F32, tag=f"b{ct}", name=f"b{ct}") for ct in range(2)]
        bpcol = [CP.tile([128, 1], F32, tag=f"bp{ct}", name=f"bp{ct}") for ct in range(2)]
        G = CP.tile([128, 8], F32, tag="G", name="G")
        Gt = CP.tile([8, 128], F32, tag="Gt", name="Gt")
        nc.sync.dma_start(out=G, in_=g_d[:, :])
        nc.sync.dma_start(out=Gt, in_=gt_d[:, :])
        for ct in range(2):
            nc.scalar.dma_start(out=wqkvT[ct], in_=wqkvT_d[ct])
            nc.sync.dma_start(out=wprojT[ct], in_=wprojT_d[ct])
            nc.scalar.dma_start(out=xq[ct], in_=xq_d[ct])
            nc.sync.dma_start(out=gcol[ct], in_=gamma_d[ct])
            nc.sync.dma_start(out=bcol[ct], in_=beta_d[ct])
            nc.sync.dma_start(out=bpcol[ct], in_=bproj_d[ct])

        # per-piece stats overlapping the DMAs: early chunks on DVE
        # (bn_stats), late chunks on ACT (Square/Identity with accumulate)
        for p in range(4):
            for ct in range(2):
                xv = xch[ct][p].rearrange("q (j f) -> q j f", f=512)
                for j in range(2):
                    i = 2 * p + j
                    if i < NDV:
                        nc.vector.bn_stats(out=stats[ct][:, i, :],
                                           in_=xv[:, j, :])
                    else:
                        scr = WK.tile([128, 512], BF16, tag="ascr", bufs=2,
                                      name="ascr")
                        nc.scalar.activation(out=scr, in_=xv[:, j, :],
                                             func=AF.Square,
                                             accum_out=sxxa[ct][:, i - NDV:
                                                                i - NDV + 1])
                        scr2 = WK.tile([128, 512], BF16, tag="ascr", bufs=2,
                                       name="ascr2")
                        nc.scalar.activation(out=scr2, in_=xv[:, j, :],
                                             func=AF.Identity,
                                             accum_out=sxa[ct][:, i - NDV:
                                                               i - NDV + 1])

        onesb = CP.tile([128, 64], F32, tag="onesb", name="onesb")
        nc.vector.memset(onesb, 1.0)
        eshcol = CP.tile([128, 1], F32, tag="eshcol", name="eshcol")
        nc.vector.memset(eshcol, ESH)

        # ---------------- groupnorm statistics ----------------
        # merge DVE bn stats (n1 = 512*NDV elems) with ACT sums (the rest):
        # mvp = [sum(x), sum(x^2)] / N per channel
        mvp = [CP.tile([128, 2], F32, tag=f"mvp{ct}", name=f"mvp{ct}")
               for ct in range(2)]
        n1 = float(512 * NDV)
        for ct in range(2):
            mv = WK.tile([128, 2], F32, tag="bnaggr", bufs=2, name="bnaggr")
            nc.vector.bn_aggr(out=mv, in_=stats[ct])
            sxs = WK.tile([128, 2], F32, tag="sxs", bufs=2, name="sxs")
            nc.vector.tensor_tensor(out=sxs[:, 0:1], in0=sxa[ct][:, 0:1],
                                    in1=sxa[ct][:, 1:2], op=OP.add)
            nc.vector.tensor_tensor(out=sxs[:, 0:1], in0=sxs[:, 0:1],
                                    in1=sxa[ct][:, 2:3], op=OP.add)
            nc.vector.tensor_tensor(out=sxs[:, 1:2], in0=sxxa[ct][:, 0:1],
                                    in1=sxxa[ct][:, 1:2], op=OP.add)
            nc.vector.tensor_tensor(out=sxs[:, 1:2], in0=sxs[:, 1:2],
                                    in1=sxxa[ct][:, 2:3], op=OP.add)
            # mv2 = [mean1, E2_1] scaled by n1, plus ACT sums, / 4096
            mv2 = WK.tile([128, 2], F32, tag="mv2", bufs=2, name="mv2")
            nc.vector.tensor_tensor(out=mv2[:, 1:2], in0=mv[:, 0:1],
                                    in1=mv[:, 0:1], op=OP.mult)
            nc.vector.tensor_tensor(out=mv2[:, 1:2], in0=mv2[:, 1:2],
                                    in1=mv[:, 1:2], op=OP.add)
            nc.vector.tensor_copy(out=mv2[:, 0:1], in_=mv[:, 0:1])
            nc.vector.tensor_scalar(out=mv2, in0=mv2, scalar1=n1,
                                    scalar2=None, op0=OP.mult)
            nc.vector.tensor_tensor(out=mv2, in0=mv2, in1=sxs, op=OP.add)
            nc.vector.tensor_scalar(out=mvp[ct], in0=mv2, scalar1=1.0 / N,
                                    scalar2=None, op0=OP.mult)

        gg = PS.tile([8, 4], F32, tag="S", bufs=2, name="S")
        for ct in range(2):
            nc.tensor.matmul(gg[:, 2 * ct:2 * ct + 2], lhsT=G, rhs=mvp[ct],
                             start=(ct == 0), stop=(ct == 1))
        ggv = gg.rearrange("p (ct two) -> p ct two", two=2)
        meanL = CP.tile([8, 2], F32, tag="meanL", name="meanL")
        rstd = CP.tile([8, 2], F32, tag="rstd", name="rstd")
        veps = WK.tile([8, 2], F32, tag="veps", bufs=1, name="veps")
        hv = WK.tile([8, 2], F32, tag="hv", bufs=1, name="hv")
        t8 = WK.tile([8, 2], F32, tag="t8", bufs=1, name="t8")
        y8 = WK.tile([8, 2], F32, tag="y8", bufs=1, name="y8")
        cmagic = WK.tile([8, 2], I32, tag="cmagic", bufs=1, name="cmagic")
        nc.vector.memset(cmagic, 0x5F3759DF)
        nc.vector.tensor_scalar(out=meanL, in0=ggv[:, :, 0], scalar1=1.0 / GS,
                                scalar2=None, op0=OP.mult)
        nc.vector.tensor_scalar(out=veps, in0=ggv[:, :, 1], scalar1=1.0 / GS,
                                scalar2=None, op0=OP.mult)
        nc.vector.tensor_tensor(out=t8, in0=meanL, in1=meanL, op=OP.mult)
        nc.vector.tensor_tensor(out=veps, in0=veps, in1=t8, op=OP.subtract)
        nc.vector.tensor_scalar(out=veps, in0=veps, scalar1=EPS, scalar2=None,
                                op0=OP.add)
        # quake rsqrt: y0 = bitcast(C - (bits(v) >> 1)), 2 Newton steps
        nc.vector.tensor_scalar(out=hv, in0=veps, scalar1=0.5, scalar2=None,
                                op0=OP.mult)
        ivi = WK.tile([8, 2], I32, tag="ivi", bufs=1, name="ivi")
        nc.vector.tensor_scalar(out=ivi, in0=veps.bitcast(I32), scalar1=1,
                                scalar2=None, op0=OP.arith_shift_right)
        nc.vector.tensor_tensor(out=y8.bitcast(I32), in0=cmagic, in1=ivi,
                                op=OP.subtract)
        for _ in range(2):
            nc.vector.tensor_tensor(out=t8, in0=y8, in1=y8, op=OP.mult)
            nc.vector.tensor_tensor(out=t8, in0=t8, in1=hv, op=OP.mult)
            nc.vector.tensor_scalar(out=t8, in0=t8, scalar1=-1.0, scalar2=1.5,
                                    op0=OP.mult, op1=OP.add)
            nc.vector.tensor_tensor(out=y8, in0=y8, in1=t8, op=OP.mult)
        nc.vector.tensor_copy(out=rstd, in_=y8)

        # expand group values to channels and build a, b~
        acol = [CP.tile([128, 1], F32, tag=f"acol{ct}", name=f"acol{ct}")
                for ct in range(2)]
        btcol = [CP.tile([128, 1], BF16, tag=f"btcol{ct}", name=f"btcol{ct}")
                 for ct in range(2)]
        for ct in range(2):
            rexp = PS.tile([128, 1], F32, tag="S", bufs=2, name="S")
            nc.tensor.matmul(rexp, lhsT=Gt, rhs=rstd[:, ct:ct + 1],
                             start=True, stop=True)
            mexp = PS.tile([128, 1], F32, tag="S", bufs=2, name="S")
            nc.tensor.matmul(mexp, lhsT=Gt, rhs=meanL[:, ct:ct + 1],
                             start=True, stop=True)
            nc.vector.tensor_tensor(out=acol[ct], in0=rexp, in1=gcol[ct],
                                    op=OP.mult)
            bwk = WK.tile([128, 1], F32, tag="bwk", bufs=2, name="bwk")
            nc.vector.tensor_tensor(out=bwk, in0=mexp, in1=acol[ct],
                                    op=OP.mult)
            nc.vector.tensor_tensor(out=btcol[ct], in0=bcol[ct], in1=bwk,
                                    op=OP.subtract)

        # scaled weights W'^T = W^T * a (per-partition), bf16
        wqs = [CP.tile([128, 3 * C], BF16, tag=f"wqs{ct}", name=f"wqs{ct}")
               for ct in range(2)]
        for ct in range(2):
            nc.vector.tensor_scalar_mul(out=wqs[ct], in0=wqkvT[ct],
                                        scalar1=acol[ct])

        # qkv bias beta = W^T.T @ b~  (q blocks 0,1 and v blocks 4,5)
        betaq = CP.tile([128, 2], F32, tag="betaq", name="betaq")
        betav = CP.tile([128, 2], BF16, tag="betav", name="betav")
        for i, ob in enumerate((0, 1, 4, 5)):
            bps = PS.tile([128, 1], F32, tag="S", bufs=2, name="S")
            for ct in range(2):
                nc.tensor.matmul(bps, lhsT=wqkvT[ct][:, 128 * ob:128 * (ob + 1)],
                                 rhs=btcol[ct], start=(ct == 0), stop=(ct == 1))
            dst = betaq if ob < 2 else betav
            nc.vector.tensor_copy(out=dst[:, i % 2:i % 2 + 1], in_=bps)

        # B* = b_proj + W_p @ beta_v
        bstar = CP.tile([128, 2], F32, tag="bstar", name="bstar")
        for ob in range(2):
            bps = PS.tile([128, 1], F32, tag="S", bufs=2, name="S")
            for ct in range(2):
                nc.tensor.matmul(bps,
                                 lhsT=wprojT[ct][:, 128 * ob:128 * (ob + 1)],
                                 rhs=betav[:, ct:ct + 1],
                                 start=(ct == 0), stop=(ct == 1))
            nc.vector.tensor_tensor(out=bstar[:, ob:ob + 1], in0=bps,
                                    in1=bpcol[ob], op=OP.add)

        # per-head projection weights at partitions 0-63
        wps4 = [None] * HEADS
        for h in range(HEADS):
            if h % 2 == 0:
                wps4[h] = wprojT[h // 2][0:64, :]
            else:
                wodd = CP.tile([64, C], BF16, tag=f"wps4_{h}", name=f"wps4_{h}")
                nc.sync.dma_start(out=wodd, in_=wprojT[h // 2][64:128, :])
                wps4[h] = wodd

        # ---------------- q (this core's slice) ----------------
        q = [CP.tile([128, TSL], BF16, tag=f"q{ob}", name=f"q{ob}")
             for ob in range(2)]
        for ob in range(2):
            qps = PS.tile([128, TSL], F32, tag="S", bufs=2, name="S")
            for ct in range(2):
                nc.tensor.matmul(qps,
                                 lhsT=wqs[ct][:, 128 * ob:128 * (ob + 1)],
                                 rhs=xq[ct], start=(ct == 0), stop=(ct == 1))
            nc.vector.tensor_scalar(out=q[ob], in0=qps,
                                    scalar1=betaq[:, ob:ob + 1], scalar2=None,
                                    op0=OP.add)

        # ---------------- k, v^T production ----------------
        # k for an ob-block lands in [128, 1024] double-j chunks
        kc = [[CP.tile([128, 1024], BF16, tag=f"k{ob}_{jp}", name=f"k{ob}_{jp}")
               for jp in range(4)] for ob in range(2)]
        # fp8 v^T for even heads (u = h//2), DoubleRow pair layout:
        #   vt8[j][p, P, c, u, w]  (P = jj pair, c = jj in pair, u = head//2)
        vt8 = [CP.tile([128, 2, 2, 2, VP8], FP8, tag=f"vt8_{j}", name=f"vt8_{j}")
               for j in range(8)]
        # bf16 v^T for odd heads: vtb[j][p, jj, u, w]
        vtb = [CP.tile([128, 4, 2, VW], BF16, tag=f"vtb{j}", name=f"vtb{j}")
               for j in range(8)]
        for j in range(8):
            nc.vector.memset(vt8[j][:, :, :, :, D:D + 1], 1.0)
            nc.vector.memset(vtb[j][:, :, :, D:VW], 1.0)

        def xb_slice(j, ct, width, off=0):
            p = j // 2
            col = 512 * (j % 2) + off
            return xch[ct][p][:, col:col + width]

        def produce_k2(jp, ob, pool, tag):
            # two j-chunks (1024 s positions) in one go
            kps = pool.tile([128, 1024], F32, tag=tag, bufs=1, name="kps")
            for half in range(2):
                for ct in range(2):
                    nc.tensor.matmul(
                        kps[:, 512 * half:512 * (half + 1)],
                        lhsT=wqs[ct][:, C + 128 * ob: C + 128 * (ob + 1)],
                        rhs=xch[ct][jp][:, 512 * half:512 * (half + 1)],
                        start=(ct == 0), stop=(ct == 1))
            nc.scalar.copy(out=kc[ob][jp], in_=kps)

        def produce_k1(jp, half, ob, pool, tag):
            # single 512-wide chunk (fits a one-bank PSUM slot in pass 2)
            kps = pool.tile([128, 512], F32, tag=tag, bufs=1, name="kps")
            for ct in range(2):
                nc.tensor.matmul(
                    kps,
                    lhsT=wqs[ct][:, C + 128 * ob: C + 128 * (ob + 1)],
                    rhs=xch[ct][jp][:, 512 * half:512 * (half + 1)],
                    start=(ct == 0), stop=(ct == 1))
            nc.scalar.copy(out=kc[ob][jp][:, 512 * half:512 * (half + 1)],
                           in_=kps)

        def kslice(ob, j, jj, po, ch=None):
            # [64, 128] d-rows x s-cols piece for the QK matmul
            jp, jr = j // 2, j % 2
            base = 64 * po
            col = 512 * jr + 128 * jj
            return kc[ob][jp][base:base + 64, col:col + 128]

        def produce_v(j, pool, tag):
            vps = pool.tile([128, 4, C], F32, tag=tag, bufs=1, name="vps")
            for jj in range(4):
                for ct in range(2):
                    nc.tensor.matmul(
                        vps[:, jj, :],
                        lhsT=xb_slice(j, ct, 128, off=128 * jj),
                        rhs=wqs[ct][:, 2 * C:3 * C],
                        start=(ct == 0), stop=(ct == 1))
            # [p, jj, (u v d)] with u=even/odd head pair, v=parity, d=64
            vsrc = vps.rearrange("p jj (u v d) -> p jj u v d", u=2, v=2)
            v8 = vt8[j].rearrange("p P c u w -> p (P c) u w")
            nc.vector.tensor_copy(out=v8[:, :, :, 0:D], in_=vsrc[:, :, :, 0, :])
            nc.scalar.copy(out=vtb[j][:, :, :, 0:D],
                           in_=vsrc[:, :, :, 1, :])

        hp = [None] * HEADS

        def attention_pass(pair, prod_hook, hpA, hpB):
            # software-pipelined two levels deep: QK(sb+1) is emitted before
            # the exps of sb, and the AV matmuls consume exp outputs one step
            # (one pair for fp8) late so the PE never waits on ACT/DVE.
            sps_t = [None, None]
            pt8_t = [None, None]
            ptb_t = [None, None, None]

            def qk(sb):
                j, jj = sb // 4, sb % 4
                sps = PS.tile([128, 2 * TSL], F32, tag="S", bufs=2, name="S")
                for po in range(2):
                    nc.tensor.matmul(
                        sps[:, TSL * po:TSL * (po + 1)],
                        lhsT=kslice(pair, j, jj, po),
                        rhs=q[pair][64 * po:64 * po + 64, :],
                        start=True, stop=True)
                sps_t[sb % 2] = sps

            def av_b(sb):
                j, jj = sb // 4, sb % 4
                nc.tensor.matmul(
                    hpB,
                    lhsT=vtb[j][:, jj, pair, :],
                    rhs=ptb_t[sb % 3], start=(sb == 0), stop=(sb == NSB - 1))

            def av_a(pr):
                # pair pr covers s-blocks (2pr, 2pr+1)
                j, jjp = pr // 2, pr % 2
                nc.tensor.matmul(
                    hpA,
                    lhsT=vt8[j][:, jjp, :, pair, 0:VW],
                    rhs=pt8_t[pr % 2],
                    perf_mode=PM.DoubleRow,
                    start=(pr == 0), stop=(pr == NSB // 2 - 1))

            qk(0)
            for sb in range(NSB):
                if sb + 1 < NSB:
                    qk(sb + 1)
                prod_hook(sb)
                sps = sps_t[sb % 2]
                if sb % 2 == 0:
                    pt8_t[(sb // 2) % 2] = P8Pool.tile([128, 2, TSL], FP8,
                                                       tag="P8", name="P8")
                pt8 = pt8_t[(sb // 2) % 2]
                # even head: exact exp(s-2) -> fp8 on ACT
                nc.scalar.activation(out=pt8[:, sb % 2, :], in_=sps[:, 0:TSL],
                                     func=AF.Exp, bias=eshcol[:, 0:1])
                # odd head: Schraudolph exp -> bf16 bits on DVE
                ptb = PBPool.tile([128, TSL], BF16, tag="PB", name="PB")
                nc.vector.tensor_scalar(out=ptb.bitcast(I16),
                                        in0=sps[:, TSL:2 * TSL],
                                        scalar1=SCH_A, scalar2=SCH_B,
                                        op0=OP.mult, op1=OP.add)
                ptb_t[sb % 3] = ptb
                # delayed AV consumption
                if sb >= 1:
                    av_b(sb - 1)
                if sb >= 2 and sb % 2 == 0:
                    av_a(sb // 2 - 1)
            av_b(NSB - 1)
            av_a(NSB // 2 - 1)

        rs = [WK.tile([VW, TSL], F32, tag=f"rs{h}", bufs=1, name=f"rs{h}")
              for h in range(HEADS)]
        bb = [WK.tile([64, TSL], F32, tag=f"bb{h}", bufs=1, name=f"bb{h}")
              for h in range(HEADS)]
        hn = [WK.tile([64, TSL], BF16, tag=f"hn{h}", bufs=1, name=f"hn{h}")
              for h in range(HEADS)]

        def normalize_head(h):
            # 1/rowsum on DVE (base partition 0: HW quirk with custom ops),
            # broadcast via a K=1 matmul (DMA broadcast has ~6us latency),
            # then multiply out of PSUM.
            nc.vector.reciprocal_approx_fast(out=rs[h][0:D + 1, :],
                                             in_=hp[h][0:D + 1, :])
            bps = PS.tile([64, TSL], F32, tag="S", bufs=2, name="S")
            nc.tensor.matmul(bps, lhsT=onesb[D:D + 1, :],
                             rhs=rs[h][D:D + 1, :], start=True, stop=True)
            nc.scalar.copy(out=bb[h], in_=bps)
            nc.vector.tensor_tensor(out=hn[h], in0=hp[h][0:D, :], in1=bb[h],
                                    op=OP.mult)

        # ---------------- pass 1: heads 0,1 (+ all k/v production) --------
        hp[0] = PS.tile([VW, TSL], F32, tag="h0", name="h0")
        hp[1] = PS.tile([VW, TSL], F32, tag="h1", name="h1")
        with tc.tile_pool(name="prod", bufs=1, space="PSUM") as PROD:
            produce_k2(0, 0, PROD, "prod")
            produce_v(0, PROD, "prod")
            produce_v(1, PROD, "prod")

            def hook1(sb):
                j, jj = sb // 4, sb % 4
                if jj == 1 and j % 2 == 0 and j < 6:
                    produce_k2(j // 2 + 1, 0, PROD, "prod")
                elif jj == 2 and j < 6:
                    produce_v(j + 2, PROD, "prod")
                elif jj == 3 and j >= 6:
                    produce_k2(j - 6, 1, PROD, "prod")

            attention_pass(0, hook1, hp[0], hp[1])

        # normalization of heads 0,1 overlaps pass 2
        for h in range(2):
            normalize_head(h)

        # ---------------- pass 2: heads 2,3 ----------------
        with tc.tile_pool(name="psB", bufs=1, space="PSUM") as PSB:
            hp[2] = PSB.tile([VW, TSL], F32, tag="h2", name="h2")
            hp[3] = PSB.tile([VW, TSL], F32, tag="h3", name="h3")

            def hook2(sb):
                j, jj = sb // 4, sb % 4
                if jj == 1 and j < 4:
                    produce_k1(2 + j // 2, j % 2, 1, PS, f"h{j % 2}")

            attention_pass(1, hook2, hp[2], hp[3])

            # ---------------- tail: heads 2,3 + projection ----------------
            for h in (2, 3):
                normalize_head(h)

            outsb = [CP.tile([128, TSL], F32, tag=f"o{ob}", name=f"o{ob}")
                     for ob in range(2)]
            for ob in range(2):
                ops = PS.tile([128, TSL], F32, tag="S", bufs=2, name="S")
                for h in range(HEADS):
                    nc.tensor.matmul(ops,
                                     lhsT=wps4[h][:, 128 * ob:128 * (ob + 1)],
                                     rhs=hn[h], start=(h == 0),
                                     stop=(h == HEADS - 1))
                nc.vector.scalar_tensor_tensor(out=outsb[ob], in0=ops,
                                               scalar=bstar[:, ob:ob + 1],
                                               in1=xq[ob], op0=OP.add,
                                               op1=OP.add)
                nc.sync.dma_start(out=out_d[ob], in_=outsb[ob])


_CACHE = {}


def _get_module():
    if "nc" not in _CACHE:
        _CACHE["nc"] = _build()
    return _CACHE["nc"]


def _bf16(a):
    return np.ascontiguousarray(a.astype(ml_dtypes.bfloat16))


def kernel(x, gn_gamma, gn_beta, w_qkv, w_proj, b_proj):
    x = np.ascontiguousarray(np.asarray(x, dtype=np.float32))
    gn_gamma = np.asarray(gn_gamma, dtype=np.float32)
    gn_beta = np.asarray(gn_beta, dtype=np.float32)
    w_qkv = np.asarray(w_qkv, dtype=np.float32)
    w_proj = np.asarray(w_proj, dtype=np.float32)
    b_proj = np.asarray(b_proj, dtype=np.float32)

    B, Cc, H, W, Dd = x.shape
    x2 = x.reshape(Cc, H * W * Dd)

    # reference splits qkv per head: rows [192h,192h+64) = q_h, then k_h, v_h.
    # Permute to [q_all | k_all | v_all] with head-major 64-row blocks.
    perm = np.concatenate([np.arange(4) * 192 + 64 * p + np.arange(64)[:, None]
                           for p in range(3)], axis=1).T.reshape(-1)
    wqkvT = np.ascontiguousarray(w_qkv.T[:, perm]).copy()
    wqkvT[:, 0:C] *= 1.0 / np.sqrt(float(D))   # fold logit scale into q
    wprojT = np.ascontiguousarray(w_proj.T)

    # group-membership indicator matrices (constant)
    ch = np.arange(128)
    gmat = (ch[:, None] // GS == np.arange(8)[None, :]).astype(np.float32)
    gtmat = np.ascontiguousarray(gmat.T)

    x2b = _bf16(x2)
    base = {
        "xbf": np.ascontiguousarray(x2b.reshape(2, 128, N)),
        "wqkvT": _bf16(wqkvT).reshape(2, 128, 3 * C),
        "wprojT": _bf16(wprojT).reshape(2, 128, C),
        "gamma_col": np.ascontiguousarray(gn_gamma.reshape(2, 128, 1)),
        "beta_col": np.ascontiguousarray(gn_beta.reshape(2, 128, 1)),
        "bproj_col": np.ascontiguousarray(b_proj.reshape(2, 128, 1)),
        "gmat": np.ascontiguousarray(gmat),
        "gtmat": gtmat,
    }
    in_maps = []
    for i in range(NCORES):
        m = dict(base)
        m["xq"] = np.ascontiguousarray(
            x2b[:, i * TSL:(i + 1) * TSL].reshape(2, 128, TSL))
        in_maps.append(m)

    nc = _get_module()
    res = run_bass_kernel_spmd(nc, in_maps, core_ids=list(range(NCORES)),
                               **_CACHE.get("run_kwargs", {}))
    _CACHE["last_result"] = res
    out = np.concatenate(
        [res.results[i]["out"].reshape(C, TSL) for i in range(NCORES)], axis=1)
    return out.reshape(B, Cc, H, W, Dd).astype(np.float32)


# revision 22
# speedup vs baseline: 1.0165x; 1.0165x over previous
"""Fused GroupNorm + attention block for Trainium2 (8 NeuronCores, SPMD).

v3 strategy:
  - Each core computes the full output for 1/8 of the spatial positions
    (a 512-column slice of the flattened [C=256, N=4096] activation).
  - x is cast to bf16 on the host (halves the HBM load, no device casts);
    GroupNorm stats run on the bf16 copy; the residual uses bf16 x.
  - GroupNorm folded into the QKV weights on-device; rstd via a
    quake-style rsqrt on the DVE so ACT only ever runs Exp (one table).
  - Attention in "S^T" layout; per head-pair step, softmax exp splits
    across engines: ACT computes exp(s-2) into fp8(e4m3) for the even
    head (shift keeps the range inside fp8; it cancels in normalization),
    the DVE computes the odd head via a Schraudolph exp (one tensor_scalar
    writing bf16 bits as int16, ~3.3% element error that washes out).
  - Even-head AV runs as fp8 DoubleRow matmuls contracting two s-blocks
    (256 positions) per instruction; odd-head AV stays bf16.
  - The attention loop is software-pipelined: the next step's S^T matmul
    is emitted before this step's AV so the PE never waits on exp.
  - Row-sums ride along as an all-ones column of v^T; 1/rowsum via
    reciprocal_approx_fast (base partition 0 - HW quirk), broadcast to
    64 partitions with a tiny K=1 matmul instead of a DMA (the DMA
    completion latency was ~6us).
"""

import numpy as np
import ml_dtypes

import concourse.bass as bass
import concourse.bacc as bacc
import concourse.tile as tile
import concourse.mybir as mybir
from concourse.bass_utils import run_bass_kernel_spmd

F32 = mybir.dt.float32
BF16 = mybir.dt.bfloat16
FP8 = mybir.dt.float8e4
I32 = mybir.dt.int32
I16 = mybir.dt.int16
AF = mybir.ActivationFunctionType
OP = mybir.AluOpType
PM = mybir.MatmulPerfMode

C = 256
N = 4096
NCORES = 8
TSL = N // NCORES          # 512 spatial positions per core
HEADS = 4
D = 64                     # head dim
NG = 16                    # groupnorm groups
GS = C // NG               # channels per group
EPS = 1e-5
NSB = N // 128             # 32 s-blocks
VW = D + 1                 # v^T columns per head incl. ones column
VP8 = 80                   # fp8 v^T padded width (pair step must be %16)
ESH = -2.0                 # logit shift for the fp8 exp path

# Schraudolph exp -> bf16 bits: bits_i16 = trunc(x * SCH_A + SCH_B)
SCH_A = 128.0 / float(np.log(2.0))     # 184.6650558736922
SCH_B = 127.0 * 128.0 - 5.0            # calibrated for truncation


def _build():
    nc = bacc.Bacc("TRN2", target_bir_lowering=False, debug=False,
                   num_devices=NCORES)

    x_d = nc.dram_tensor("xbf", [2, 128, N], BF16, kind="ExternalInput")
    xq_d = nc.dram_tensor("xq", [2, 128, TSL], BF16, kind="ExternalInput")
    wqkvT_d = nc.dram_tensor("wqkvT", [2, 128, 3 * C], BF16, kind="ExternalInput")
    wprojT_d = nc.dram_tensor("wprojT", [2, 128, C], BF16, kind="ExternalInput")
    gamma_d = nc.dram_tensor("gamma_col", [2, 128, 1], F32, kind="ExternalInput")
    beta_d = nc.dram_tensor("beta_col", [2, 128, 1], F32, kind="ExternalInput")
    bproj_d = nc.dram_tensor("bproj_col", [2, 128, 1], F32, kind="ExternalInput")
    g_d = nc.dram_tensor("gmat", [128, NG // 2], F32, kind="ExternalInput")
    gt_d = nc.dram_tensor("gtmat", [NG // 2, 128], F32, kind="ExternalInput")
    out_d = nc.dram_tensor("out", [2, 128, TSL], F32, kind="ExternalOutput")

    with tile.TileContext(nc) as tc:
        _emit(nc, tc, x_d, xq_d, wqkvT_d, wprojT_d, gamma_d, beta_d,
              bproj_d, g_d, gt_d, out_d)
    nc.finalize()
    return nc


def _emit(nc, tc, x_d, xq_d, wqkvT_d, wprojT_d, gamma_d, beta_d, bproj_d,
          g_d, gt_d, out_d):
    import contextlib
    ctx = contextlib.ExitStack()
    with ctx:
        CP = ctx.enter_context(tc.tile_pool(name="const", bufs=1))
        WK = ctx.enter_context(tc.tile_pool(name="work", bufs=2))
        PS = ctx.enter_context(tc.tile_pool(name="psum", bufs=1, space="PSUM"))
        P8Pool = ctx.enter_context(tc.tile_pool(name="p8tiles", bufs=2))
        PBPool = ctx.enter_context(tc.tile_pool(name="pbtiles", bufs=3))

        # ---------------- loads (x bf16 in 8 pieces, two DMA rings) ------
        xch = [[CP.tile([128, N // 4], BF16, tag=f"x{ct}{p}", name=f"x{ct}{p}")
                for p in range(4)] for ct in range(2)]
        NDV = 5            # chunks handled by DVE bn_stats; rest go to ACT
        stats = [WK.tile([128, NDV, 6], F32, tag=f"bnstats{ct}", bufs=1,
                         name=f"bnstats{ct}") for ct in range(2)]
        sxa = [WK.tile([128, 8 - NDV], F32, tag=f"sxa{ct}", bufs=1,
                       name=f"sxa{ct}") for ct in range(2)]
        sxxa = [WK.tile([128, 8 - NDV], F32, tag=f"sxxa{ct}", bufs=1,
                        name=f"sxxa{ct}") for ct in range(2)]
        for p in range(4):
            for ct in range(2):
                eng = nc.sync if ct == 0 else nc.scalar
                eng.dma_start(out=xch[ct][p],
                              in_=x_d[ct, :, p * (N // 4):(p + 1) * (N // 4)])
        xq = [CP.tile([128, TSL], BF16, tag=f"xq{ct}", name=f"xq{ct}")
              for ct in range(2)]
        wqkvT = [CP.tile([128, 3 * C], BF16, tag=f"wq{ct}", name=f"wq{ct}")
                 for ct in range(2)]
        wprojT = [CP.tile([128, C], BF16, tag=f"wp{ct}", name=f"wp{ct}")
                  for ct in range(2)]
        gcol = [CP.tile([128, 1], F32, tag=f"g{ct}", name=f"g{ct}") for ct in range(2)]
        bcol = [CP.tile([128, 1], F32, tag=f"b{ct}", name=f"b{ct}") for ct in range(2)]
        bpcol = [CP.tile([128, 1], F32, tag=f"bp{ct}", name=f"bp{ct}") for ct in range(2)]
        G = CP.tile([128, 8], F32, tag="G", name="G")
        Gt = CP.tile([8, 128], F32, tag="Gt", name="Gt")
        nc.sync.dma_start(out=G, in_=g_d[:, :])
        nc.sync.dma_start(out=Gt, in_=gt_d[:, :])
        for ct in range(2):
            nc.scalar.dma_start(out=wqkvT[ct], in_=wqkvT_d[ct])
            nc.sync.dma_start(out=wprojT[ct], in_=wprojT_d[ct])
            nc.scalar.dma_start(out=xq[ct], in_=xq_d[ct])
            nc.sync.dma_start(out=gcol[ct], in_=gamma_d[ct])
            nc.sync.dma_start(out=bcol[ct], in_=beta_d[ct])
            nc.sync.dma_start(out=bpcol[ct], in_=bproj_d[ct])

        # per-piece stats overlapping the DMAs: early chunks on DVE
        # (bn_stats), late chunks on ACT (Square/Identity with accumulate)
        for p in range(4):
            for ct in range(2):
                xv = xch[ct][p].rearrange("q (j f) -> q j f", f=512)
                for j in range(2):
                    i = 2 * p + j
                    if i < NDV:
                        nc.vector.bn_stats(out=stats[ct][:, i, :],
                                           in_=xv[:, j, :])
                    else:
                        scr = WK.tile([128, 512], BF16, tag="ascr", bufs=2,
                                      name="ascr")
                        nc.scalar.activation(out=scr, in_=xv[:, j, :],
                                             func=AF.Square,
                                             accum_out=sxxa[ct][:, i - NDV:
                                                                i - NDV + 1])
                        scr2 = WK.tile([128, 512], BF16, tag="ascr", bufs=2,
                                       name="ascr2")
                        nc.scalar.activation(out=scr2, in_=xv[:, j, :],
                                             func=AF.Identity,
                                             accum_out=sxa[ct][:, i - NDV:
                                                               i - NDV + 1])

        onesb = CP.tile([128, 64], F32, tag="onesb", name="onesb")
        nc.vector.memset(onesb, 1.0)
        eshcol = CP.tile([128, 1], F32, tag="eshcol", name="eshcol")
        nc.vector.memset(eshcol, ESH)

        # ---------------- groupnorm statistics ----------------
        # merge DVE bn stats (n1 = 512*NDV elems) with ACT sums (the rest):
        # mvp = [sum(x), sum(x^2)] / N per channel
        mvp = [CP.tile([128, 2], F32, tag=f"mvp{ct}", name=f"mvp{ct}")
               for ct in range(2)]
        n1 = float(512 * NDV)
        for ct in range(2):
            mv = WK.tile([128, 2], F32, tag="bnaggr", bufs=2, name="bnaggr")
            nc.vector.bn_aggr(out=mv, in_=stats[ct])
            sxs = WK.tile([128, 2], F32, tag="sxs", bufs=2, name="sxs")
            nc.vector.tensor_tensor(out=sxs[:, 0:1], in0=sxa[ct][:, 0:1],
                                    in1=sxa[ct][:, 1:2], op=OP.add)
            nc.vector.tensor_tensor(out=sxs[:, 0:1], in0=sxs[:, 0:1],
                                    in1=sxa[ct][:, 2:3], op=OP.add)
            nc.vector.tensor_tensor(out=sxs[:, 1:2], in0=sxxa[ct][:, 0:1],
                                    in1=sxxa[ct][:, 1:2], op=OP.add)
            nc.vector.tensor_tensor(out=sxs[:, 1:2], in0=sxs[:, 1:2],
                                    in1=sxxa[ct][:, 2:3], op=OP.add)
            # mv2 = [mean1, E2_1] scaled by n1, plus ACT sums, / 4096
            mv2 = WK.tile([128, 2], F32, tag="mv2", bufs=2, name="mv2")
            nc.vector.tensor_tensor(out=mv2[:, 1:2], in0=mv[:, 0:1],
                                    in1=mv[:, 0:1], op=OP.mult)
            nc.vector.tensor_tensor(out=mv2[:, 1:2], in0=mv2[:, 1:2],
                                    in1=mv[:, 1:2], op=OP.add)
            nc.vector.tensor_copy(out=mv2[:, 0:1], in_=mv[:, 0:1])
            nc.vector.tensor_scalar(out=mv2, in0=mv2, scalar1=n1,
                                    scalar2=None, op0=OP.mult)
            nc.vector.tensor_tensor(out=mv2, in0=mv2, in1=sxs, op=OP.add)
            nc.vector.tensor_scalar(out=mvp[ct], in0=mv2, scalar1=1.0 / N,
                                    scalar2=None, op0=OP.mult)

        gg = PS.tile([8, 4], F32, tag="S", bufs=2, name="S")
        for ct in range(2):
            nc.tensor.matmul(gg[:, 2 * ct:2 * ct + 2], lhsT=G, rhs=mvp[ct],
                             start=(ct == 0), stop=(ct == 1))
        ggv = gg.rearrange("p (ct two) -> p ct two", two=2)
        meanL = CP.tile([8, 2], F32, tag="meanL", name="meanL")
        rstd = CP.tile([8, 2], F32, tag="rstd", name="rstd")
        veps = WK.tile([8, 2], F32, tag="veps", bufs=1, name="veps")
        hv = WK.tile([8, 2], F32, tag="hv", bufs=1, name="hv")
        t8 = WK.tile([8, 2], F32, tag="t8", bufs=1, name="t8")
        y8 = WK.tile([8, 2], F32, tag="y8", bufs=1, name="y8")
        cmagic = WK.tile([8, 2], I32, tag="cmagic", bufs=1, name="cmagic")
        nc.vector.memset(cmagic, 0x5F3759DF)
        nc.vector.tensor_scalar(out=meanL, in0=ggv[:, :, 0], scalar1=1.0 / GS,
                                scalar2=None, op0=OP.mult)
        nc.vector.tensor_scalar(out=veps, in0=ggv[:, :, 1], scalar1=1.0 / GS,
                                scalar2=None, op0=OP.mult)
        nc.vector.tensor_tensor(out=t8, in0=meanL, in1=meanL, op=OP.mult)
        nc.vector.tensor_tensor(out=veps, in0=veps, in1=t8, op=OP.subtract)
        nc.vector.tensor_scalar(out=veps, in0=veps, scalar1=EPS, scalar2=None,
                                op0=OP.add)
        # quake rsqrt: y0 = bitcast(C - (bits(v) >> 1)), 2 Newton steps
        nc.vector.tensor_scalar(out=hv, in0=veps, scalar1=0.5, scalar2=None,
                                op0=OP.mult)
        ivi = WK.tile([8, 2], I32, tag="ivi", bufs=1, name="ivi")
        nc.vector.tensor_scalar(out=ivi, in0=veps.bitcast(I32), scalar1=1,
                                scalar2=None, op0=OP.arith_shift_right)
        nc.vector.tensor_tensor(out=y8.bitcast(I32), in0=cmagic, in1=ivi,
                                op=OP.subtract)
        for _ in range(2):
            nc.vector.tensor_tensor(out=t8, in0=y8, in1=y8, op=OP.mult)
            nc.vector.tensor_tensor(out=t8, in0=t8, in1=hv, op=OP.mult)
            nc.vector.tensor_scalar(out=t8, in0=t8, scalar1=-1.0, scalar2=1.5,
                                    op0=OP.mult, op1=OP.add)
            nc.vector.tensor_tensor(out=y8, in0=y8, in1=t8, op=OP.mult)
        nc.vector.tensor_copy(out=rstd, in_=y8)

        # expand group values to channels and build a, b~
        acol = [CP.tile([128, 1], F32, tag=f"acol{ct}", name=f"acol{ct}")
                for ct in range(2)]
        btcol = [CP.tile([128, 1], BF16, tag=f"btcol{ct}", name=f"btcol{ct}")
                 for ct in range(2)]
        for ct in range(2):
            rexp = PS.tile([128, 1], F32, tag="S", bufs=2, name="S")
            nc.tensor.matmul(rexp, lhsT=Gt, rhs=rstd[:, ct:ct + 1],
                             start=True, stop=True)
            mexp = PS.tile([128, 1], F32, tag="S", bufs=2, name="S")
            nc.tensor.matmul(mexp, lhsT=Gt, rhs=meanL[:, ct:ct + 1],
                             start=True, stop=True)
            nc.vector.tensor_tensor(out=acol[ct], in0=rexp, in1=gcol[ct],
                                    op=OP.mult)
            bwk = WK.tile([128, 1], F32, tag="bwk", bufs=2, name="bwk")
            nc.vector.tensor_tensor(out=bwk, in0=mexp, in1=acol[ct],
                                    op=OP.mult)
            nc.vector.tensor_tensor(out=btcol[ct], in0=bcol[ct], in1=bwk,
                                    op=OP.subtract)

        # scaled weights W'^T = W^T * a (per-partition), bf16
        wqs = [CP.tile([128, 3 * C], BF16, tag=f"wqs{ct}", name=f"wqs{ct}")
               for ct in range(2)]
        for ct in range(2):
            nc.vector.tensor_scalar_mul(out=wqs[ct], in0=wqkvT[ct],
                                        scalar1=acol[ct])

        # qkv bias beta = W^T.T @ b~  (q blocks 0,1 and v blocks 4,5)
        betaq = CP.tile([128, 2], F32, tag="betaq", name="betaq")
        betav = CP.tile([128, 2], BF16, tag="betav", name="betav")
        for i, ob in enumerate((0, 1, 4, 5)):
            bps = PS.tile([128, 1], F32, tag="S", bufs=2, name="S")
            for ct in range(2):
                nc.tensor.matmul(bps, lhsT=wqkvT[ct][:, 128 * ob:128 * (ob + 1)],
                                 rhs=btcol[ct], start=(ct == 0), stop=(ct == 1))
            dst = betaq if ob < 2 else betav
            nc.vector.tensor_copy(out=dst[:, i % 2:i % 2 + 1], in_=bps)

        # B* = b_proj + W_p @ beta_v
        bstar = CP.tile([128, 2], F32, tag="bstar", name="bstar")
        for ob in range(2):
            bps = PS.tile([128, 1], F32, tag="S", bufs=2, name="S")
            for ct in range(2):
                nc.tensor.matmul(bps,
                                 lhsT=wprojT[ct][:, 128 * ob:128 * (ob + 1)],
                                 rhs=betav[:, ct:ct + 1],
                                 start=(ct == 0), stop=(ct == 1))
            nc.vector.tensor_tensor(out=bstar[:, ob:ob + 1], in0=bps,
                                    in1=bpcol[ob], op=OP.add)

        # per-head projection weights at partitions 0-63
        wps4 = [None] * HEADS
        for h in range(HEADS):
            if h % 2 == 0:
                wps4[h] = wprojT[h // 2][0:64, :]
            else:
                wodd = CP.tile([64, C], BF16, tag=f"wps4_{h}", name=f"wps4_{h}")
                nc.sync.dma_start(out=wodd, in_=wprojT[h // 2][64:128, :])
                wps4[h] = wodd

        # ---------------- q (this core's slice) ----------------
        q = [CP.tile([128, TSL], BF16, tag=f"q{ob}", name=f"q{ob}")
             for ob in range(2)]
        for ob in range(2):
            qps = PS.tile([128, TSL], F32, tag="S", bufs=2, name="S")
            for ct in range(2):
                nc.tensor.matmul(qps,
                                 lhsT=wqs[ct][:, 128 * ob:128 * (ob + 1)],
                                 rhs=xq[ct], start=(ct == 0), stop=(ct == 1))
            nc.vector.tensor_scalar(out=q[ob], in0=qps,
                                    scalar1=betaq[:, ob:ob + 1], scalar2=None,
                                    op0=OP.add)

        # ---------------- k, v^T production ----------------
        # k for an ob-block lands in [128, 1024] double-j chunks
        kc = [[CP.tile([128, 1024], BF16, tag=f"k{ob}_{jp}", name=f"k{ob}_{jp}")
               for jp in range(4)] for ob in range(2)]
        # fp8 v^T for even heads (u = h//2), DoubleRow pair layout:
        #   vt8[j][p, P, c, u, w]  (P = jj pair, c = jj in pair, u = head//2)
        vt8 = [CP.tile([128, 2, 2, 2, VP8], FP8, tag=f"vt8_{j}", name=f"vt8_{j}")
               for j in range(8)]
        # bf16 v^T for odd heads: vtb[j][p, jj, u, w]
        vtb = [CP.tile([128, 4, 2, VW], BF16, tag=f"vtb{j}", name=f"vtb{j}")
               for j in range(8)]
        for j in range(8):
            nc.vector.memset(vt8[j][:, :, :, :, D:D + 1], 1.0)
            nc.vector.memset(vtb[j][:, :, :, D:VW], 1.0)

        def xb_slice(j, ct, width, off=0):
            p = j // 2
            col = 512 * (j % 2) + off
            return xch[ct][p][:, col:col + width]

        def produce_k2(jp, ob, pool, tag):
            # two j-chunks (1024 s positions) in one go
            kps = pool.tile([128, 1024], F32, tag=tag, bufs=1, name="kps")
            for half in range(2):
                for ct in range(2):
                    nc.tensor.matmul(
                        kps[:, 512 * half:512 * (half + 1)],
                        lhsT=wqs[ct][:, C + 128 * ob: C + 128 * (ob + 1)],
                        rhs=xch[ct][jp][:, 512 * half:512 * (half + 1)],
                        start=(ct == 0), stop=(ct == 1))
            nc.scalar.copy(out=kc[ob][jp], in_=kps)

        def produce_k1(jp, half, ob, pool, tag):
            # single 512-wide chunk (fits a one-bank PSUM slot in pass 2)
            kps = pool.tile([128, 512], F32, tag=tag, bufs=1, name="kps")
            for ct in range(2):
                nc.tensor.matmul(
                    kps,
                    lhsT=wqs[ct][:, C + 128 * ob: C + 128 * (ob + 1)],
                    rhs=xch[ct][jp][:, 512 * half:512 * (half + 1)],
                    start=(ct == 0), stop=(ct == 1))
            nc.scalar.copy(out=kc[ob][jp][:, 512 * half:512 * (half + 1)],
                           in_=kps)

        def kslice(ob, j, jj, po, ch=None):
            # [64, 128] d-rows x s-cols piece for the QK matmul
            jp, jr = j // 2, j % 2
            base = 64 * po
            col = 512 * jr + 128 * jj
            return kc[ob][jp][base:base + 64, col:col + 128]

        def produce_v(j, pool, tag):
            vps = pool.tile([128, 4, C], F32, tag=tag, bufs=1, name="vps")
            for jj in range(4):
                for ct in range(2):
                    nc.tensor.matmul(
                        vps[:, jj, :],
                        lhsT=xb_slice(j, ct, 128, off=128 * jj),
                        rhs=wqs[ct][:, 2 * C:3 * C],
                        start=(ct == 0), stop=(ct == 1))
            # [p, jj, (u v d)] with u=even/odd head pair, v=parity, d=64
            vsrc = vps.rearrange("p jj (u v d) -> p jj u v d", u=2, v=2)
            v8 = vt8[j].rearrange("p P c u w -> p (P c) u w")
            nc.vector.tensor_copy(out=v8[:, :, :, 0:D], in_=vsrc[:, :, :, 0, :])
            nc.vector.tensor_copy(out=vtb[j][:, :, :, 0:D],
                                  in_=vsrc[:, :, :, 1, :])

        hp = [None] * HEADS

        def attention_pass(pair, prod_hook, hpA, hpB):
            # software-pipelined two levels deep: QK(sb+1) is emitted before
            # the exps of sb, and the AV matmuls consume exp outputs one step
            # (one pair for fp8) late so the PE never waits on ACT/DVE.
            sps_t = [None, None]
            pt8_t = [None, None]
            ptb_t = [None, None, None]

            def qk(sb):
                j, jj = sb // 4, sb % 4
                sps = PS.tile([128, 2 * TSL], F32, tag="S", bufs=2, name="S")
                for po in range(2):
                    nc.tensor.matmul(
                        sps[:, TSL * po:TSL * (po + 1)],
                        lhsT=kslice(pair, j, jj, po),
                        rhs=q[pair][64 * po:64 * po + 64, :],
                        start=True, stop=True)
                sps_t[sb % 2] = sps

            def av_b(sb):
                j, jj = sb // 4, sb % 4
                nc.tensor.matmul(
                    hpB,
                    lhsT=vtb[j][:, jj, pair, :],
                    rhs=ptb_t[sb % 3], start=(sb == 0), stop=(sb == NSB - 1))

            def av_a(pr):
                # pair pr covers s-blocks (2pr, 2pr+1)
                j, jjp = pr // 2, pr % 2
                nc.tensor.matmul(
                    hpA,
                    lhsT=vt8[j][:, jjp, :, pair, 0:VW],
                    rhs=pt8_t[pr % 2],
                    perf_mode=PM.DoubleRow,
                    start=(pr == 0), stop=(pr == NSB // 2 - 1))

            qk(0)
            for sb in range(NSB):
                if sb + 1 < NSB:
                    qk(sb + 1)
                prod_hook(sb)
                sps = sps_t[sb % 2]
                if sb % 2 == 0:
                    pt8_t[(sb // 2) % 2] = P8Pool.tile([128, 2, TSL], FP8,
                                                       tag="P8", name="P8")
                pt8 = pt8_t[(sb // 2) % 2]
                # even head: exact exp(s-2) -> fp8 on ACT
                nc.scalar.activation(out=pt8[:, sb % 2, :], in_=sps[:, 0:TSL],
                                     func=AF.Exp, bias=eshcol[:, 0:1])
                # odd head: Schraudolph exp -> bf16 bits on DVE
                ptb = PBPool.tile([128, TSL], BF16, tag="PB", name="PB")
                nc.vector.tensor_scalar(out=ptb.bitcast(I16),
                                        in0=sps[:, TSL:2 * TSL],
                                        scalar1=SCH_A, scalar2=SCH_B,
                                        op0=OP.mult, op1=OP.add)
                ptb_t[sb % 3] = ptb
                # delayed AV consumption
                if sb >= 1:
                    av_b(sb - 1)
                if sb >= 2 and sb % 2 == 0:
                    av_a(sb // 2 - 1)
            av_b(NSB - 1)
            av_a(NSB // 2 - 1)

        rs = [WK.tile([VW, TSL], F32, tag=f"rs{h}", bufs=1, name=f"rs{h}")
              for h in range(HEADS)]
        bb = [WK.tile([64, TSL], F32, tag=f"bb{h}", bufs=1, name=f"bb{h}")
              for h in range(HEADS)]
        hn = [WK.tile([64, TSL], BF16, tag=f"hn{h}", bufs=1, name=f"hn{h}")
              for h in range(HEADS)]

        def normalize_head(h):
            # 1/rowsum on DVE (base partition 0: HW quirk with custom ops),
            # broadcast via a K=1 matmul (DMA broadcast has ~6us latency),
            # then multiply out of PSUM.
            nc.vector.reciprocal_approx_fast(out=rs[h][0:D + 1, :],
                                             in_=hp[h][0:D + 1, :])
            bps = PS.tile([64, TSL], F32, tag="S", bufs=2, name="S")
            nc.tensor.matmul(bps, lhsT=onesb[D:D + 1, :],
                             rhs=rs[h][D:D + 1, :], start=True, stop=True)
            nc.scalar.copy(out=bb[h], in_=bps)
            nc.vector.tensor_tensor(out=hn[h], in0=hp[h][0:D, :], in1=bb[h],
                                    op=OP.mult)

        # ---------------- pass 1: heads 0,1 (+ all k/v production) --------
        hp[0] = PS.tile([VW, TSL], F32, tag="h0", name="h0")
        hp[1] = PS.tile([VW, TSL], F32, tag="h1", name="h1")
        with tc.tile_pool(name="prod", bufs=1, space="PSUM") as PROD:
            produce_k2(0, 0, PROD, "prod")
            produce_v(0, PROD, "prod")
            produce_v(1, PROD, "prod")

            def hook1(sb):
                j, jj = sb // 4, sb % 4
                if jj == 1 and j % 2 == 0 and j < 6:
                    produce_k2(j // 2 + 1, 0, PROD, "prod")
                elif jj == 2 and j < 6:
                    produce_v(j + 2, PROD, "prod")
                elif jj == 3 and j >= 6:
                    produce_k2(j - 6, 1, PROD, "prod")

            attention_pass(0, hook1, hp[0], hp[1])

        # normalization of heads 0,1 overlaps pass 2
        for h in range(2):
            normalize_head(h)

        # ---------------- pass 2: heads 2,3 ----------------
        with tc.tile_pool(name="psB", bufs=1, space="PSUM") as PSB:
            hp[2] = PSB.tile([VW, TSL], F32, tag="h2", name="h2")
            hp[3] = PSB.tile([VW, TSL], F32, tag="h3", name="h3")

            def hook2(sb):
                j, jj = sb // 4, sb % 4
                if jj == 1 and j < 4:
                    produce_k1(2 + j // 2, j % 2, 1, PS, f"h{j % 2}")

            attention_pass(1, hook2, hp[2], hp[3])

            # ---------------- tail: heads 2,3 + projection ----------------
            for h in (2, 3):
                normalize_head(h)

            outsb = [CP.tile([128, TSL], F32, tag=f"o{ob}", name=f"o{ob}")
                     for ob in range(2)]
            for ob in range(2):
                ops = PS.tile([128, TSL], F32, tag="S", bufs=2, name="S")
                for h in range(HEADS):
                    nc.tensor.matmul(ops,
                                     lhsT=wps4[h][:, 128 * ob:128 * (ob + 1)],
                                     rhs=hn[h], start=(h == 0),
                                     stop=(h == HEADS - 1))
                nc.vector.scalar_tensor_tensor(out=outsb[ob], in0=ops,
                                               scalar=bstar[:, ob:ob + 1],
                                               in1=xq[ob], op0=OP.add,
                                               op1=OP.add)
                nc.sync.dma_start(out=out_d[ob], in_=outsb[ob])


_CACHE = {}


def _get_module():
    if "nc" not in _CACHE:
        _CACHE["nc"] = _build()
    return _CACHE["nc"]


def _bf16(a):
    return np.ascontiguousarray(a.astype(ml_dtypes.bfloat16))


def kernel(x, gn_gamma, gn_beta, w_qkv, w_proj, b_proj):
    x = np.ascontiguousarray(np.asarray(x, dtype=np.float32))
    gn_gamma = np.asarray(gn_gamma, dtype=np.float32)
    gn_beta = np.asarray(gn_beta, dtype=np.float32)
    w_qkv = np.asarray(w_qkv, dtype=np.float32)
    w_proj = np.asarray(w_proj, dtype=np.float32)
    b_proj = np.asarray(b_proj, dtype=np.float32)

    B, Cc, H, W, Dd = x.shape
    x2 = x.reshape(Cc, H * W * Dd)

    # reference splits qkv per head: rows [192h,192h+64) = q_h, then k_h, v_h.
    # Permute to [q_all | k_all | v_all] with head-major 64-row blocks.
    perm = np.concatenate([np.arange(4) * 192 + 64 * p + np.arange(64)[:, None]
                           for p in range(3)], axis=1).T.reshape(-1)
    wqkvT = np.ascontiguousarray(w_qkv.T[:, perm]).copy()
    wqkvT[:, 0:C] *= 1.0 / np.sqrt(float(D))   # fold logit scale into q
    wprojT = np.ascontiguousarray(w_proj.T)

    # group-membership indicator matrices (constant)
    ch = np.arange(128)
    gmat = (ch[:, None] // GS == np.arange(8)[None, :]).astype(np.float32)
    gtmat = np.ascontiguousarray(gmat.T)

    x2b = _bf16(x2)
    base = {
        "xbf": np.ascontiguousarray(x2b.reshape(2, 128, N)),
        "wqkvT": _bf16(wqkvT).reshape(2, 128, 3 * C),
        "wprojT": _bf16(wprojT).reshape(2, 128, C),
        "gamma_col": np.ascontiguousarray(gn_gamma.reshape(2, 128, 1)),
        "beta_col": np.ascontiguousarray(gn_beta.reshape(2, 128, 1)),
        "bproj_col": np.ascontiguousarray(b_proj.reshape(2, 128, 1)),
        "gmat": np.ascontiguousarray(gmat),
        "gtmat": gtmat,
    }
    in_maps = []
    for i in range(NCORES):
        m = dict(base)
        m["xq"] = np.ascontiguousarray(
            x2b[:, i * TSL:(i + 1) * TSL].reshape(2, 128, TSL))
        in_maps.append(m)

    nc = _get_module()
    res = run_bass_kernel_spmd(nc, in_maps, core_ids=list(range(NCORES)),
                               **_CACHE.get("run_kwargs", {}))
    _CACHE["last_result"] = res
    out = np.concatenate(
        [res.results[i]["out"].reshape(C, TSL) for i in range(NCORES)], axis=1)
    return out.reshape(B, Cc, H, W, Dd).astype(np.float32)


# revision 23
# speedup vs baseline: 1.0424x; 1.0255x over previous
"""Fused GroupNorm + attention block for Trainium2 (8 NeuronCores, SPMD).

v3 strategy:
  - Each core computes the full output for 1/8 of the spatial positions
    (a 512-column slice of the flattened [C=256, N=4096] activation).
  - x is cast to bf16 on the host (halves the HBM load, no device casts);
    GroupNorm stats run on the bf16 copy; the residual uses bf16 x.
  - GroupNorm folded into the QKV weights on-device; rstd via a
    quake-style rsqrt on the DVE so ACT only ever runs Exp (one table).
  - Attention in "S^T" layout; per head-pair step, softmax exp splits
    across engines: ACT computes exp(s-2) into fp8(e4m3) for the even
    head (shift keeps the range inside fp8; it cancels in normalization),
    the DVE computes the odd head via a Schraudolph exp (one tensor_scalar
    writing bf16 bits as int16, ~3.3% element error that washes out).
  - Even-head AV runs as fp8 DoubleRow matmuls contracting two s-blocks
    (256 positions) per instruction; odd-head AV stays bf16.
  - The attention loop is software-pipelined: the next step's S^T matmul
    is emitted before this step's AV so the PE never waits on exp.
  - Row-sums ride along as an all-ones column of v^T; 1/rowsum via
    reciprocal_approx_fast (base partition 0 - HW quirk), broadcast to
    64 partitions with a tiny K=1 matmul instead of a DMA (the DMA
    completion latency was ~6us).
"""

import numpy as np
import ml_dtypes

import concourse.bass as bass
import concourse.bacc as bacc
import concourse.tile as tile
import concourse.mybir as mybir
from concourse.bass_utils import run_bass_kernel_spmd

F32 = mybir.dt.float32
BF16 = mybir.dt.bfloat16
FP8 = mybir.dt.float8e4
I32 = mybir.dt.int32
I16 = mybir.dt.int16
AF = mybir.ActivationFunctionType
OP = mybir.AluOpType
PM = mybir.MatmulPerfMode

C = 256
N = 4096
NCORES = 8
TSL = N // NCORES          # 512 spatial positions per core
HEADS = 4
D = 64                     # head dim
NG = 16                    # groupnorm groups
GS = C // NG               # channels per group
EPS = 1e-5
NSB = N // 128             # 32 s-blocks
VW = D + 1                 # v^T columns per head incl. ones column
VP8 = 80                   # fp8 v^T padded width (pair step must be %16)
ESH = -2.0                 # logit shift for the fp8 exp path

# Schraudolph exp -> bf16 bits: bits_i16 = trunc(x * SCH_A + SCH_B)
SCH_A = 128.0 / float(np.log(2.0))     # 184.6650558736922
SCH_B = 127.0 * 128.0 - 5.0            # calibrated for truncation


def _build():
    nc = bacc.Bacc("TRN2", target_bir_lowering=False, debug=False,
                   num_devices=NCORES)

    x_d = nc.dram_tensor("xbf", [2, 128, N], BF16, kind="ExternalInput")
    xq_d = nc.dram_tensor("xq", [2, 128, TSL], BF16, kind="ExternalInput")
    wqkvT_d = nc.dram_tensor("wqkvT", [2, 128, 3 * C], BF16, kind="ExternalInput")
    wprojT_d = nc.dram_tensor("wprojT", [2, 128, C], BF16, kind="ExternalInput")
    gamma_d = nc.dram_tensor("gamma_col", [2, 128, 1], F32, kind="ExternalInput")
    beta_d = nc.dram_tensor("beta_col", [2, 128, 1], F32, kind="ExternalInput")
    bproj_d = nc.dram_tensor("bproj_col", [2, 128, 1], F32, kind="ExternalInput")
    g_d = nc.dram_tensor("gmat", [128, NG // 2], F32, kind="ExternalInput")
    gt_d = nc.dram_tensor("gtmat", [NG // 2, 128], F32, kind="ExternalInput")
    out_d = nc.dram_tensor("out", [2, 128, TSL], F32, kind="ExternalOutput")

    with tile.TileContext(nc) as tc:
        _emit(nc, tc, x_d, xq_d, wqkvT_d, wprojT_d, gamma_d, beta_d,
              bproj_d, g_d, gt_d, out_d)
    nc.finalize()
    return nc


def _emit(nc, tc, x_d, xq_d, wqkvT_d, wprojT_d, gamma_d, beta_d, bproj_d,
          g_d, gt_d, out_d):
    import contextlib
    ctx = contextlib.ExitStack()
    with ctx:
        CP = ctx.enter_context(tc.tile_pool(name="const", bufs=1))
        WK = ctx.enter_context(tc.tile_pool(name="work", bufs=2))
        PS = ctx.enter_context(tc.tile_pool(name="psum", bufs=1, space="PSUM"))
        P8Pool = ctx.enter_context(tc.tile_pool(name="p8tiles", bufs=2))
        PBPool = ctx.enter_context(tc.tile_pool(name="pbtiles", bufs=3))

        # ---------------- loads (x bf16 in 8 pieces, two DMA rings) ------
        xch = [[CP.tile([128, N // 4], BF16, tag=f"x{ct}{p}", name=f"x{ct}{p}")
                for p in range(4)] for ct in range(2)]
        stats = [WK.tile([128, 8, 6], F32, tag=f"bnstats{ct}", bufs=1,
                         name=f"bnstats{ct}") for ct in range(2)]
        for p in range(4):
            for ct in range(2):
                eng = nc.sync if ct == 0 else nc.scalar
                eng.dma_start(out=xch[ct][p],
                              in_=x_d[ct, :, p * (N // 4):(p + 1) * (N // 4)])
        xq = [CP.tile([128, TSL], BF16, tag=f"xq{ct}", name=f"xq{ct}")
              for ct in range(2)]
        wqkvT = [CP.tile([128, 3 * C], BF16, tag=f"wq{ct}", name=f"wq{ct}")
                 for ct in range(2)]
        wprojT = [CP.tile([128, C], BF16, tag=f"wp{ct}", name=f"wp{ct}")
                  for ct in range(2)]
        gcol = [CP.tile([128, 1], F32, tag=f"g{ct}", name=f"g{ct}") for ct in range(2)]
        bcol = [CP.tile([128, 1], F32, tag=f"b{ct}", name=f"b{ct}") for ct in range(2)]
        bpcol = [CP.tile([128, 1], F32, tag=f"bp{ct}", name=f"bp{ct}") for ct in range(2)]
        G = CP.tile([128, 8], F32, tag="G", name="G")
        Gt = CP.tile([8, 128], F32, tag="Gt", name="Gt")
        nc.sync.dma_start(out=G, in_=g_d[:, :])
        nc.sync.dma_start(out=Gt, in_=gt_d[:, :])
        for ct in range(2):
            nc.scalar.dma_start(out=wqkvT[ct], in_=wqkvT_d[ct])
            nc.sync.dma_start(out=wprojT[ct], in_=wprojT_d[ct])
            nc.scalar.dma_start(out=xq[ct], in_=xq_d[ct])
            nc.sync.dma_start(out=gcol[ct], in_=gamma_d[ct])
            nc.sync.dma_start(out=bcol[ct], in_=beta_d[ct])
            nc.sync.dma_start(out=bpcol[ct], in_=bproj_d[ct])

        # per-piece bn_stats (DVE), overlapping the DMAs
        for p in range(4):
            for ct in range(2):
                xv = xch[ct][p].rearrange("q (j f) -> q j f", f=512)
                for j in range(2):
                    nc.vector.bn_stats(out=stats[ct][:, 2 * p + j, :],
                                       in_=xv[:, j, :])

        onesb = CP.tile([128, 64], F32, tag="onesb", name="onesb")
        nc.vector.memset(onesb, 1.0)
        eshcol = CP.tile([128, 1], F32, tag="eshcol", name="eshcol")
        nc.vector.memset(eshcol, ESH)

        # ---------------- groupnorm statistics ----------------
        mvp = [CP.tile([128, 2], F32, tag=f"mvp{ct}", name=f"mvp{ct}")
               for ct in range(2)]
        for ct in range(2):
            mv = WK.tile([128, 2], F32, tag="bnaggr", bufs=2, name="bnaggr")
            nc.vector.bn_aggr(out=mv, in_=stats[ct])
            nc.vector.tensor_copy(out=mvp[ct][:, 0:1], in_=mv[:, 0:1])
            nc.vector.tensor_tensor(out=mvp[ct][:, 1:2], in0=mv[:, 0:1],
                                    in1=mv[:, 0:1], op=OP.mult)
            nc.vector.tensor_tensor(out=mvp[ct][:, 1:2], in0=mvp[ct][:, 1:2],
                                    in1=mv[:, 1:2], op=OP.add)

        gg = PS.tile([8, 4], F32, tag="S", bufs=2, name="S")
        for ct in range(2):
            nc.tensor.matmul(gg[:, 2 * ct:2 * ct + 2], lhsT=G, rhs=mvp[ct],
                             start=(ct == 0), stop=(ct == 1))
        ggv = gg.rearrange("p (ct two) -> p ct two", two=2)
        meanL = CP.tile([8, 2], F32, tag="meanL", name="meanL")
        rstd = CP.tile([8, 2], F32, tag="rstd", name="rstd")
        veps = WK.tile([8, 2], F32, tag="veps", bufs=1, name="veps")
        hv = WK.tile([8, 2], F32, tag="hv", bufs=1, name="hv")
        t8 = WK.tile([8, 2], F32, tag="t8", bufs=1, name="t8")
        y8 = WK.tile([8, 2], F32, tag="y8", bufs=1, name="y8")
        cmagic = WK.tile([8, 2], I32, tag="cmagic", bufs=1, name="cmagic")
        nc.vector.memset(cmagic, 0x5F3759DF)
        nc.vector.tensor_scalar(out=meanL, in0=ggv[:, :, 0], scalar1=1.0 / GS,
                                scalar2=None, op0=OP.mult)
        nc.vector.tensor_scalar(out=veps, in0=ggv[:, :, 1], scalar1=1.0 / GS,
                                scalar2=None, op0=OP.mult)
        nc.vector.tensor_tensor(out=t8, in0=meanL, in1=meanL, op=OP.mult)
        nc.vector.tensor_tensor(out=veps, in0=veps, in1=t8, op=OP.subtract)
        nc.vector.tensor_scalar(out=veps, in0=veps, scalar1=EPS, scalar2=None,
                                op0=OP.add)
        # quake rsqrt: y0 = bitcast(C - (bits(v) >> 1)), 2 Newton steps
        nc.vector.tensor_scalar(out=hv, in0=veps, scalar1=0.5, scalar2=None,
                                op0=OP.mult)
        ivi = WK.tile([8, 2], I32, tag="ivi", bufs=1, name="ivi")
        nc.vector.tensor_scalar(out=ivi, in0=veps.bitcast(I32), scalar1=1,
                                scalar2=None, op0=OP.arith_shift_right)
        nc.vector.tensor_tensor(out=y8.bitcast(I32), in0=cmagic, in1=ivi,
                                op=OP.subtract)
        for _ in range(2):
            nc.vector.tensor_tensor(out=t8, in0=y8, in1=y8, op=OP.mult)
            nc.vector.tensor_tensor(out=t8, in0=t8, in1=hv, op=OP.mult)
            nc.vector.tensor_scalar(out=t8, in0=t8, scalar1=-1.0, scalar2=1.5,
                                    op0=OP.mult, op1=OP.add)
            nc.vector.tensor_tensor(out=y8, in0=y8, in1=t8, op=OP.mult)
        nc.vector.tensor_copy(out=rstd, in_=y8)

        # expand group values to channels and build a, b~
        acol = [CP.tile([128, 1], F32, tag=f"acol{ct}", name=f"acol{ct}")
                for ct in range(2)]
        btcol = [CP.tile([128, 1], BF16, tag=f"btcol{ct}", name=f"btcol{ct}")
                 for ct in range(2)]
        for ct in range(2):
            rexp = PS.tile([128, 1], F32, tag="S", bufs=2, name="S")
            nc.tensor.matmul(rexp, lhsT=Gt, rhs=rstd[:, ct:ct + 1],
                             start=True, stop=True)
            mexp = PS.tile([128, 1], F32, tag="S", bufs=2, name="S")
            nc.tensor.matmul(mexp, lhsT=Gt, rhs=meanL[:, ct:ct + 1],
                             start=True, stop=True)
            nc.vector.tensor_tensor(out=acol[ct], in0=rexp, in1=gcol[ct],
                                    op=OP.mult)
            bwk = WK.tile([128, 1], F32, tag="bwk", bufs=2, name="bwk")
            nc.vector.tensor_tensor(out=bwk, in0=mexp, in1=acol[ct],
                                    op=OP.mult)
            nc.vector.tensor_tensor(out=btcol[ct], in0=bcol[ct], in1=bwk,
                                    op=OP.subtract)

        # scaled weights W'^T = W^T * a (per-partition), bf16
        wqs = [CP.tile([128, 3 * C], BF16, tag=f"wqs{ct}", name=f"wqs{ct}")
               for ct in range(2)]
        for ct in range(2):
            nc.vector.tensor_scalar_mul(out=wqs[ct], in0=wqkvT[ct],
                                        scalar1=acol[ct])

        # qkv bias beta = W^T.T @ b~  (q blocks 0,1 and v blocks 4,5)
        betaq = CP.tile([128, 2], F32, tag="betaq", name="betaq")
        betav = CP.tile([128, 2], BF16, tag="betav", name="betav")
        for i, ob in enumerate((0, 1, 4, 5)):
            bps = PS.tile([128, 1], F32, tag="S", bufs=2, name="S")
            for ct in range(2):
                nc.tensor.matmul(bps, lhsT=wqkvT[ct][:, 128 * ob:128 * (ob + 1)],
                                 rhs=btcol[ct], start=(ct == 0), stop=(ct == 1))
            dst = betaq if ob < 2 else betav
            nc.vector.tensor_copy(out=dst[:, i % 2:i % 2 + 1], in_=bps)

        # B* = b_proj + W_p @ beta_v
        bstar = CP.tile([128, 2], F32, tag="bstar", name="bstar")
        for ob in range(2):
            bps = PS.tile([128, 1], F32, tag="S", bufs=2, name="S")
            for ct in range(2):
                nc.tensor.matmul(bps,
                                 lhsT=wprojT[ct][:, 128 * ob:128 * (ob + 1)],
                                 rhs=betav[:, ct:ct + 1],
                                 start=(ct == 0), stop=(ct == 1))
            nc.vector.tensor_tensor(out=bstar[:, ob:ob + 1], in0=bps,
                                    in1=bpcol[ob], op=OP.add)

        # per-head projection weights at partitions 0-63
        wps4 = [None] * HEADS
        for h in range(HEADS):
            if h % 2 == 0:
                wps4[h] = wprojT[h // 2][0:64, :]
            else:
                wodd = CP.tile([64, C], BF16, tag=f"wps4_{h}", name=f"wps4_{h}")
                nc.sync.dma_start(out=wodd, in_=wprojT[h // 2][64:128, :])
                wps4[h] = wodd

        # ---------------- q (this core's slice) ----------------
        q = [CP.tile([128, TSL], BF16, tag=f"q{ob}", name=f"q{ob}")
             for ob in range(2)]
        for ob in range(2):
            qps = PS.tile([128, TSL], F32, tag="S", bufs=2, name="S")
            for ct in range(2):
                nc.tensor.matmul(qps,
                                 lhsT=wqs[ct][:, 128 * ob:128 * (ob + 1)],
                                 rhs=xq[ct], start=(ct == 0), stop=(ct == 1))
            nc.vector.tensor_scalar(out=q[ob], in0=qps,
                                    scalar1=betaq[:, ob:ob + 1], scalar2=None,
                                    op0=OP.add)

        # ---------------- k, v^T production ----------------
        # k for an ob-block lands in [128, 1024] double-j chunks
        kc = [[CP.tile([128, 1024], BF16, tag=f"k{ob}_{jp}", name=f"k{ob}_{jp}")
               for jp in range(4)] for ob in range(2)]
        # fp8 v^T for even heads (u = h//2), DoubleRow pair layout:
        #   vt8[j][p, P, c, u, w]  (P = jj pair, c = jj in pair, u = head//2)
        vt8 = [CP.tile([128, 2, 2, 2, VP8], FP8, tag=f"vt8_{j}", name=f"vt8_{j}")
               for j in range(8)]
        # bf16 v^T for odd heads: vtb[j][p, jj, u, w]
        vtb = [CP.tile([128, 4, 2, VW], BF16, tag=f"vtb{j}", name=f"vtb{j}")
               for j in range(8)]
        for j in range(8):
            nc.vector.memset(vt8[j][:, :, :, :, D:D + 1], 1.0)
            nc.vector.memset(vtb[j][:, :, :, D:VW], 1.0)

        def xb_slice(j, ct, width, off=0):
            p = j // 2
            col = 512 * (j % 2) + off
            return xch[ct][p][:, col:col + width]

        def produce_k2(jp, ob, pool, tag):
            # two j-chunks (1024 s positions) in one go
            kps = pool.tile([128, 1024], F32, tag=tag, bufs=1, name="kps")
            for half in range(2):
                for ct in range(2):
                    nc.tensor.matmul(
                        kps[:, 512 * half:512 * (half + 1)],
                        lhsT=wqs[ct][:, C + 128 * ob: C + 128 * (ob + 1)],
                        rhs=xch[ct][jp][:, 512 * half:512 * (half + 1)],
                        start=(ct == 0), stop=(ct == 1))
            nc.scalar.copy(out=kc[ob][jp], in_=kps)

        def produce_k1(jp, half, ob, pool, tag):
            # single 512-wide chunk (fits a one-bank PSUM slot in pass 2)
            kps = pool.tile([128, 512], F32, tag=tag, bufs=1, name="kps")
            for ct in range(2):
                nc.tensor.matmul(
                    kps,
                    lhsT=wqs[ct][:, C + 128 * ob: C + 128 * (ob + 1)],
                    rhs=xch[ct][jp][:, 512 * half:512 * (half + 1)],
                    start=(ct == 0), stop=(ct == 1))
            nc.scalar.copy(out=kc[ob][jp][:, 512 * half:512 * (half + 1)],
                           in_=kps)

        def kslice(ob, j, jj, po, ch=None):
            # [64, 128] d-rows x s-cols piece for the QK matmul
            jp, jr = j // 2, j % 2
            base = 64 * po
            col = 512 * jr + 128 * jj
            return kc[ob][jp][base:base + 64, col:col + 128]

        def produce_v(j, pool, tag):
            vps = pool.tile([128, 4, C], F32, tag=tag, bufs=1, name="vps")
            for jj in range(4):
                for ct in range(2):
                    nc.tensor.matmul(
                        vps[:, jj, :],
                        lhsT=xb_slice(j, ct, 128, off=128 * jj),
                        rhs=wqs[ct][:, 2 * C:3 * C],
                        start=(ct == 0), stop=(ct == 1))
            # [p, jj, (u v d)] with u=even/odd head pair, v=parity, d=64
            vsrc = vps.rearrange("p jj (u v d) -> p jj u v d", u=2, v=2)
            v8 = vt8[j].rearrange("p P c u w -> p (P c) u w")
            nc.vector.tensor_copy(out=v8[:, :, :, 0:D], in_=vsrc[:, :, :, 0, :])
            nc.vector.tensor_copy(out=vtb[j][:, :, :, 0:D],
                                  in_=vsrc[:, :, :, 1, :])

        hp = [None] * HEADS

        def attention_pass(pair, prod_hook, hpA, hpB):
            # software-pipelined two levels deep: QK(sb+1) is emitted before
            # the exps of sb, and the AV matmuls consume exp outputs one step
            # (one pair for fp8) late so the PE never waits on ACT/DVE.
            sps_t = [None, None]
            pt8_t = [None, None]
            ptb_t = [None, None, None]

            def qk(sb):
                j, jj = sb // 4, sb % 4
                sps = PS.tile([128, 2 * TSL], F32, tag="S", bufs=2, name="S")
                for po in range(2):
                    nc.tensor.matmul(
                        sps[:, TSL * po:TSL * (po + 1)],
                        lhsT=kslice(pair, j, jj, po),
                        rhs=q[pair][64 * po:64 * po + 64, :],
                        start=True, stop=True)
                sps_t[sb % 2] = sps

            def av_b(sb):
                j, jj = sb // 4, sb % 4
                nc.tensor.matmul(
                    hpB,
                    lhsT=vtb[j][:, jj, pair, :],
                    rhs=ptb_t[sb % 3], start=(sb == 0), stop=(sb == NSB - 1))

            def av_a(pr):
                # pair pr covers s-blocks (2pr, 2pr+1)
                j, jjp = pr // 2, pr % 2
                nc.tensor.matmul(
                    hpA,
                    lhsT=vt8[j][:, jjp, :, pair, 0:VW],
                    rhs=pt8_t[pr % 2],
                    perf_mode=PM.DoubleRow,
                    start=(pr == 0), stop=(pr == NSB // 2 - 1))

            qk(0)
            for sb in range(NSB):
                if sb + 1 < NSB:
                    qk(sb + 1)
                prod_hook(sb)
                sps = sps_t[sb % 2]
                if sb % 2 == 0:
                    pt8_t[(sb // 2) % 2] = P8Pool.tile([128, 2, TSL], FP8,
                                                       tag="P8", name="P8")
                pt8 = pt8_t[(sb // 2) % 2]
                # even head: exact exp(s-2) -> fp8 on ACT
                nc.scalar.activation(out=pt8[:, sb % 2, :], in_=sps[:, 0:TSL],
                                     func=AF.Exp, bias=eshcol[:, 0:1])
                # odd head: Schraudolph exp -> bf16 bits on DVE
                ptb = PBPool.tile([128, TSL], BF16, tag="PB", name="PB")
                nc.vector.tensor_scalar(out=ptb.bitcast(I16),
                                        in0=sps[:, TSL:2 * TSL],
                                        scalar1=SCH_A, scalar2=SCH_B,
                                        op0=OP.mult, op1=OP.add)
                ptb_t[sb % 3] = ptb
                # delayed AV consumption
                if sb >= 1:
                    av_b(sb - 1)
                if sb >= 2 and sb % 2 == 0:
                    av_a(sb // 2 - 1)
            av_b(NSB - 1)
            av_a(NSB // 2 - 1)

        rs = [WK.tile([VW, TSL], F32, tag=f"rs{h}", bufs=1, name=f"rs{h}")
              for h in range(HEADS)]
        bb = [WK.tile([64, TSL], F32, tag=f"bb{h}", bufs=1, name=f"bb{h}")
              for h in range(HEADS)]
        hn = [WK.tile([64, TSL], BF16, tag=f"hn{h}", bufs=1, name=f"hn{h}")
              for h in range(HEADS)]

        def normalize_head(h):
            # 1/rowsum on DVE (base partition 0: HW quirk with custom ops),
            # broadcast via a K=1 matmul (DMA broadcast has ~6us latency),
            # then multiply out of PSUM.
            nc.vector.reciprocal_approx_fast(out=rs[h][0:D + 1, :],
                                             in_=hp[h][0:D + 1, :])
            bps = PS.tile([64, TSL], F32, tag="S", bufs=2, name="S")
            nc.tensor.matmul(bps, lhsT=onesb[D:D + 1, :],
                             rhs=rs[h][D:D + 1, :], start=True, stop=True)
            nc.scalar.copy(out=bb[h], in_=bps)
            nc.vector.tensor_tensor(out=hn[h], in0=hp[h][0:D, :], in1=bb[h],
                                    op=OP.mult)

        # ---------------- pass 1: heads 0,1 (+ all k/v production) --------
        hp[0] = PS.tile([VW, TSL], F32, tag="h0", name="h0")
        hp[1] = PS.tile([VW, TSL], F32, tag="h1", name="h1")
        with tc.tile_pool(name="prod", bufs=1, space="PSUM") as PROD:
            produce_k2(0, 0, PROD, "prod")
            produce_v(0, PROD, "prod")
            produce_v(1, PROD, "prod")

            def hook1(sb):
                j, jj = sb // 4, sb % 4
                if jj == 1 and j % 2 == 0 and j < 6:
                    produce_k2(j // 2 + 1, 0, PROD, "prod")
                elif jj == 2 and j < 6:
                    produce_v(j + 2, PROD, "prod")
                elif jj == 3 and j >= 6:
                    produce_k2(j - 6, 1, PROD, "prod")

            attention_pass(0, hook1, hp[0], hp[1])

        # normalization of heads 0,1 overlaps pass 2
        for h in range(2):
            normalize_head(h)

        # ---------------- pass 2: heads 2,3 ----------------
        with tc.tile_pool(name="psB", bufs=1, space="PSUM") as PSB:
            hp[2] = PSB.tile([VW, TSL], F32, tag="h2", name="h2")
            hp[3] = PSB.tile([VW, TSL], F32, tag="h3", name="h3")

            def hook2(sb):
                j, jj = sb // 4, sb % 4
                if jj == 1 and j < 4:
                    produce_k1(2 + j // 2, j % 2, 1, PS, f"h{j % 2}")

            attention_pass(1, hook2, hp[2], hp[3])

            # ---------------- tail: heads 2,3 + projection ----------------
            for h in (2, 3):
                normalize_head(h)

            outsb = [CP.tile([128, TSL], F32, tag=f"o{ob}", name=f"o{ob}")
                     for ob in range(2)]
            for ob in range(2):
                ops = PS.tile([128, TSL], F32, tag="S", bufs=2, name="S")
                for h in range(HEADS):
                    nc.tensor.matmul(ops,
                                     lhsT=wps4[h][:, 128 * ob:128 * (ob + 1)],
                                     rhs=hn[h], start=(h == 0),
                                     stop=(h == HEADS - 1))
                nc.vector.scalar_tensor_tensor(out=outsb[ob], in0=ops,
                                               scalar=bstar[:, ob:ob + 1],
                                               in1=xq[ob], op0=OP.add,
                                               op1=OP.add)
                nc.sync.dma_start(out=out_d[ob], in_=outsb[ob])


_CACHE = {}


def _get_module():
    if "nc" not in _CACHE:
        _CACHE["nc"] = _build()
    return _CACHE["nc"]


def _bf16(a):
    return np.ascontiguousarray(a.astype(ml_dtypes.bfloat16))


def kernel(x, gn_gamma, gn_beta, w_qkv, w_proj, b_proj):
    x = np.ascontiguousarray(np.asarray(x, dtype=np.float32))
    gn_gamma = np.asarray(gn_gamma, dtype=np.float32)
    gn_beta = np.asarray(gn_beta, dtype=np.float32)
    w_qkv = np.asarray(w_qkv, dtype=np.float32)
    w_proj = np.asarray(w_proj, dtype=np.float32)
    b_proj = np.asarray(b_proj, dtype=np.float32)

    B, Cc, H, W, Dd = x.shape
    x2 = x.reshape(Cc, H * W * Dd)

    # reference splits qkv per head: rows [192h,192h+64) = q_h, then k_h, v_h.
    # Permute to [q_all | k_all | v_all] with head-major 64-row blocks.
    perm = np.concatenate([np.arange(4) * 192 + 64 * p + np.arange(64)[:, None]
                           for p in range(3)], axis=1).T.reshape(-1)
    wqkvT = np.ascontiguousarray(w_qkv.T[:, perm]).copy()
    wqkvT[:, 0:C] *= 1.0 / np.sqrt(float(D))   # fold logit scale into q
    wprojT = np.ascontiguousarray(w_proj.T)

    # group-membership indicator matrices (constant)
    ch = np.arange(128)
    gmat = (ch[:, None] // GS == np.arange(8)[None, :]).astype(np.float32)
    gtmat = np.ascontiguousarray(gmat.T)

    x2b = _bf16(x2)
    base = {
        "xbf": np.ascontiguousarray(x2b.reshape(2, 128, N)),
        "wqkvT": _bf16(wqkvT).reshape(2, 128, 3 * C),
        "wprojT": _bf16(wprojT).reshape(2, 128, C),
        "gamma_col": np.ascontiguousarray(gn_gamma.reshape(2, 128, 1)),
        "beta_col": np.ascontiguousarray(gn_beta.reshape(2, 128, 1)),
        "bproj_col": np.ascontiguousarray(b_proj.reshape(2, 128, 1)),
        "gmat": np.ascontiguousarray(gmat),
        "gtmat": gtmat,
    }
    in_maps = []
    for i in range(NCORES):
        m = dict(base)
        m["xq"] = np.ascontiguousarray(
            x2b[:, i * TSL:(i + 1) * TSL].reshape(2, 128, TSL))
        in_maps.append(m)

    nc = _get_module()
    res = run_bass_kernel_spmd(nc, in_maps, core_ids=list(range(NCORES)),
                               **_CACHE.get("run_kwargs", {}))
    _CACHE["last_result"] = res
    out = np.concatenate(
        [res.results[i]["out"].reshape(C, TSL) for i in range(NCORES)], axis=1)
    return out.reshape(B, Cc, H, W, Dd).astype(np.float32)
